# revision 22
# baseline (speedup 1.0000x reference)
"""Adaptive-softmax NLL on 8 TRN2 NeuronCores (Bass/Tile, SPMD data-parallel).

Strategy: shard the 4096 tokens across 8 cores (512 each). Each core computes
its tokens' full NLL (head + both tails) in bf16 on TensorE, with vocab on the
free dim and tokens on PSUM partitions; ScalarE does exp with fused free-dim
accumulation (accum_out) so the softmax denominators come out of the same pass.
Target logits are computed from host-gathered weight columns (MoE-style
dispatch done at input-prep time) as elementwise-mul + ones-matvec partition
reductions. Each core emits one partial-loss scalar; the host sums 8 scalars
and divides by N.
"""

import os
import sys
import types

import numpy as np
import ml_dtypes

BF16 = ml_dtypes.bfloat16
FP8 = ml_dtypes.float8_e4m3
W8_SCALE = 256.0

# ---- problem constants (hardcoded; kernel.py must be self-contained) ----
CUTOFF = [4000, 20000, 50000]
D = 1024
N = 4096
NCORES = 8
TOK = N // NCORES          # 512 tokens per core
NT = TOK // 128            # 4 token tiles of 128
HEAD_V = CUTOFF[0] + 2     # 4002
T0_V = CUTOFF[1] - CUTOFF[0]   # 16000
T1_V = CUTOFF[2] - CUTOFF[1]   # 30000
D1 = D // 4                # 256 tail1 bottleneck


def _chunks(v):
    out = []
    while v > 0:
        out.append(min(512, v))
        v -= out[-1]
    return out


H_CH = _chunks(HEAD_V)     # [512]*7 + [418]
T0_CH = _chunks(T0_V)      # [512]*31 + [128]
T1_CH = _chunks(T1_V)      # [512]*58 + [296]

LAST_EXEC_NS = None
_CACHE = {}


def _install_axon_profile_shim():
    """The image's antenv lacks axon_hooks; register the NTFF hook + disable
    the FishPath artifact upload so BASS_TRACE=1 profiling works locally."""
    if "antenv.axon_hooks" not in sys.modules:
        try:
            import antenv  # noqa
            mod = types.ModuleType("antenv.axon_hooks")
            _hook = [None]
            mod.set_axon_ntff_profile_hook = lambda h: _hook.__setitem__(0, h)
            mod.get_axon_ntff_profile_hook = lambda: _hook[0]
            sys.modules["antenv.axon_hooks"] = mod
            antenv.axon_hooks = mod
            from trn_agent_boot.trn_boot import _ntff_profile_via_ctypes
            mod.set_axon_ntff_profile_hook(
                _ntff_profile_via_ctypes("/opt/axon/libaxon_pjrt.so")
            )
        except Exception:
            pass
    try:
        from concourse import bass_utils
        bass_utils.upload_artifacts = lambda tmpdir: f"local:{tmpdir}"
    except Exception:
        pass


# ---------------- host-side layout helpers ----------------

def _tile_k(w):
    """[K, M] f32 -> [128, K//128, M] bf16 (partition, k-tile, free)."""
    K, M = w.shape
    kd = K // 128
    return np.ascontiguousarray(
        w.reshape(kd, 128, M).transpose(1, 0, 2)
    ).astype(BF16)


def _tile_k_f8(w, scale):
    K, M = w.shape
    kd = K // 128
    return np.ascontiguousarray(
        (w * scale).reshape(kd, 128, M).transpose(1, 0, 2)
    ).astype(FP8)


def _chunk_weights(w, chunk_sizes, dtype=BF16, scale=1.0):
    """[K, V] f32 -> [nchunk, 128, K//128, 512], zero-padded ragged."""
    K, V = w.shape
    kd = K // 128
    out = np.zeros((len(chunk_sizes), 128, kd, 512), dtype=dtype)
    c0 = 0
    for i, ncs in enumerate(chunk_sizes):
        blk = (w[:, c0:c0 + ncs] * scale).reshape(kd, 128, ncs).transpose(1, 0, 2)
        out[i, :, :, :ncs] = blk.astype(dtype)
        c0 += ncs
    return out


# ---------------- device kernel builder ----------------

H1_SCALE = 32.0  # fp8 scale for the bottleneck weights w1


def _build(use_bias):
    from concourse import bass, bacc, tile, bass_isa

    mybir = bass.mybir
    dt = mybir.dt
    bf = dt.bfloat16
    f32 = dt.float32
    f8 = dt.float8e4
    AF = mybir.ActivationFunctionType
    ALU = mybir.AluOpType
    AX = mybir.AxisListType
    DR = mybir.MatmulPerfMode.DoubleRow
    RED = bass_isa.ReduceOp

    nc = bacc.Bacc(
        "TRN2",
        target_bir_lowering=False,
        debug=False,
        enable_asserts=False,
        num_devices=NCORES,
    )

    def din(name, shape, dtype=bf):
        return nc.dram_tensor(name, list(shape), dtype, kind="ExternalInput")

    wiT_h = din("wiT", (128, 8, TOK))
    wiT8_h = din("wiT8", (NT, 128, 8, 128), dt.float8e4)
    wiT8f_h = din("wiT8f", (128, 8, TOK), dt.float8e4)
    selH_h = din("selH", (128, 8, TOK))
    sel0_h = din("sel0", (128, 8, TOK))
    sel1_h = din("sel1", (128, 2, TOK))
    bsel_h = din("bsel", (1, TOK), f32)
    m0_h = din("m0", (128, NT), f32)
    m1_h = din("m1", (128, NT), f32)
    bext_h = din("bext", (1, HEAD_V))
    hw_h = din("hw", (len(H_CH), 128, 8, 512), f8)
    w20_h = din("w20", (len(T0_CH), 128, 8, 512), f8)
    w21_h = din("w21", (len(T1_CH), 128, 2, 512), f8)
    w10_h = din("w10", (128, 8, D), f8)
    w11_h = din("w11", (128, 8, D1), f8)
    out_h = nc.dram_tensor("out", [1, 1], f32, kind="ExternalOutput")

    with tile.TileContext(nc) as tc:
        with (
            tc.tile_pool(name="const", bufs=1) as cpool,
            tc.tile_pool(name="wstream", bufs=10) as wpool,
            tc.tile_pool(name="scratch", bufs=3) as spool,
            tc.tile_pool(name="pmm", bufs=int(os.environ.get("K_PSLOTS", "2")), space=bass.MemorySpace.PSUM) as pmm,
        ):
            CPG = int(os.environ.get("K_CPG", "4"))   # chunks per macro group
            GW = 512 * CPG                             # psum banks per slot = CPG

            def groups(chunk_sizes):
                out = []
                for g0 in range(0, len(chunk_sizes), CPG):
                    cs = chunk_sizes[g0:g0 + CPG]
                    items = []
                    off = 0
                    for i, ncs in enumerate(cs):
                        items.append((g0 + i, ncs, off))
                        off += ncs
                    out.append((g0 // CPG, items, off))
                return out

            wiT = cpool.tile([128, 8, TOK], bf)
            wiT8 = []
            for j in range(NT):
                wiT8_j = cpool.tile([128, 8, 128], f8, tag=f"wiT8_{j}")
                wiT8.append(wiT8_j)
            wiT8f = cpool.tile([128, 8, TOK], f8)
            w10 = cpool.tile([128, 8, D], f8)
            w11 = cpool.tile([128, 8, D1], f8)
            selH = cpool.tile([128, 8, TOK], bf)
            sel0 = cpool.tile([128, 8, TOK], bf)
            sel1 = cpool.tile([128, 2, TOK], bf)
            bsel = cpool.tile([1, TOK], f32)
            m0sb = cpool.tile([128, NT], f32)
            m1sb = cpool.tile([128, NT], f32)
            bext = cpool.tile([1, HEAD_V], bf)
            h0T = cpool.tile([128, 8, TOK], bf)
            h1T = cpool.tile([128, 2, TOK], bf)
            h0T8 = cpool.tile([128, 8, TOK], f8)
            h1T8 = cpool.tile([128, 2, TOK], f8)
            nGH = (len(H_CH) + CPG - 1) // CPG
            nG0 = (len(T0_CH) + CPG - 1) // CPG
            nG1 = (len(T1_CH) + CPG - 1) // CPG
            seH = cpool.tile([128, NT, nGH], f32)
            se0 = cpool.tile([128, NT, nG0], f32)
            se1 = cpool.tile([128, NT, nG1], f32)
            ones_row = cpool.tile([1, 128], bf)
            macc = cpool.tile([128, TOK], f32)

            for j in range(NT):
                nc.sync.dma_start(out=wiT8[j][:], in_=wiT8_h.ap()[j])
            nc.sync.dma_start(out=bext[:], in_=bext_h[:])
            nc.vector.memset(ones_row[:], 1.0)

            def late_residents():
                for p in range(0, 128, 32):
                    nc.sync.dma_start(out=w10[p:p + 32], in_=w10_h.ap()[p:p + 32])
                nc.sync.dma_start(out=w11[:], in_=w11_h[:])
                nc.sync.dma_start(out=wiT8f[:], in_=wiT8f_h[:])
                nc.sync.dma_start(out=wiT[:], in_=wiT_h[:])
                nc.sync.dma_start(out=m0sb[:], in_=m0_h[:])
                nc.sync.dma_start(out=m1sb[:], in_=m1_h[:])
                nc.sync.dma_start(out=bsel[:], in_=bsel_h[:])

            hbase = [0]
            for ncs in H_CH:
                hbase.append(hbase[-1] + ncs)

            def group_emitter(wh, nk, lhsT8, se, items, gw, bias, split=1):
                """Returns emit(jt): matmuls + exp for one token tile of one
                macro group. Weight DMAs are issued on first use."""
                nk2 = nk // 2
                g = items[0][0] // CPG
                state = {"wts": None, "split": split}

                def emit(jt):
                    if state["wts"] is None:
                        state["wts"] = []
                        for c, ncs, off in items:
                            wt = wpool.tile([128, nk, 512], f8, tag=f"w{nk}")
                            sp = 128 // state["split"]
                            for p in range(0, 128, sp):
                                nc.sync.dma_start(out=wt[p:p + sp],
                                                  in_=wh.ap()[c, p:p + sp])
                            state["wts"].append(wt)
                    ps = pmm.tile([128, GW], f32, tag="mm")
                    for (c, ncs, off), wt in zip(items, state["wts"]):
                        for k2 in range(nk2):
                            if isinstance(lhsT8, list):
                                lt = lhsT8[jt][:, 2 * k2:2 * k2 + 2, :]
                            else:
                                lt = lhsT8[:, 2 * k2:2 * k2 + 2,
                                           jt * 128:(jt + 1) * 128]
                            nc.tensor.matmul(
                                ps[:, off:off + ncs],
                                lt,
                                wt[:, 2 * k2:2 * k2 + 2, :ncs],
                                start=(k2 == 0),
                                stop=(k2 == nk2 - 1 and bias is None),
                                perf_mode=DR,
                            )
                        if bias is not None:
                            nc.tensor.matmul(
                                ps[:, off:off + ncs],
                                ones_row[:, :],
                                bias[:, hbase[c]:hbase[c] + ncs],
                                start=False,
                                stop=True,
                            )
                    nc.scalar.activation(
                        ps[:, :gw],
                        ps[:, :gw],
                        AF.Exp,
                        scale=1.0 / W8_SCALE,
                        accum_out=se[:, jt, g:g + 1],
                    )
                return emit

            def h_thunk(w1t, hT, hT8, m):
                def emit():
                    ps = pmm.tile([128, GW], f32, tag="mm")
                    for k2 in range(4):
                        nc.tensor.matmul(
                            ps[:, :TOK],
                            w1t[:, 2 * k2:2 * k2 + 2, m * 128:(m + 1) * 128],
                            wiT8f[:, 2 * k2:2 * k2 + 2, :],
                            start=(k2 == 0),
                            stop=(k2 == 3),
                            perf_mode=DR,
                        )
                    nc.vector.tensor_scalar_mul(hT[:, m, :], ps[:, :TOK],
                                                1.0 / H1_SCALE)
                    nc.vector.tensor_scalar_mul(hT8[:, m, :], ps[:, :TOK],
                                                1.0 / H1_SCALE)
                return emit

            head_groups = groups(H_CH)
            t0_groups = groups(T0_CH)
            t1_groups = groups(T1_CH)
            bias_t = bext if use_bias else None

            # phase 1: head groups jt-interleaved with h0/h1 tiles (PE-only)
            h_thunks = [h_thunk(w10, h0T, h0T8, m) for m in range(8)]
            h_thunks += [h_thunk(w11, h1T, h1T8, m) for m in range(2)]
            for gi, (g, items, gw) in enumerate(head_groups):
                em = group_emitter(hw_h, 8, wiT8, seH, items, gw, bias_t,
                                   split=4 if gi == 0 else (2 if gi == 1 else 1))
                for jt in range(NT):
                    em(jt)
                    if gi == 0 and jt == 0:
                        late_residents()
                    if gi > 0 and h_thunks:
                        h_thunks.pop(0)()
            while h_thunks:
                h_thunks.pop(0)()

            # target-logit dots on DVE (fills gaps; no PE/PSUM involved)
            nc.sync.dma_start(out=selH[:], in_=selH_h[:])
            nc.sync.dma_start(out=sel0[:], in_=sel0_h[:])
            nc.sync.dma_start(out=sel1[:], in_=sel1_h[:])
            pieces = [(wiT, selH, 8), (h0T, sel0, 8), (h1T, sel1, 2)]
            first = True
            for a, b, nk in pieces:
                for k in range(nk):
                    mt = spool.tile([128, TOK], f32, tag="mul")
                    nc.vector.tensor_mul(mt[:], a[:, k, :], b[:, k, :])
                    if first:
                        nc.vector.tensor_copy(macc[:], mt[:])
                        first = False
                    else:
                        nc.vector.tensor_add(macc[:], macc[:], mt[:])

            # phase 2: weave t0 (PE-heavy) with t1 (ACT-heavy), then pair
            # adjacent woven groups and interleave their token tiles
            weave = []
            i0, i1 = 0, 0
            n0, n1 = len(t0_groups), len(t1_groups)
            while i0 < n0 or i1 < n1:
                pick1 = (i1 < n1) and (i0 >= n0 or (n1 - i1) * n0 >= (n0 - i0) * n1)
                if pick1:
                    g, items, gw = t1_groups[i1]
                    weave.append(group_emitter(w21_h, 2, h1T8, se1, items, gw, None))
                    i1 += 1
                else:
                    g, items, gw = t0_groups[i0]
                    weave.append(group_emitter(w20_h, 8, h0T8, se0, items, gw, None))
                    i0 += 1
            for p in range(0, len(weave) - 1, 2):
                ea, eb = weave[p], weave[p + 1]
                for jt in range(NT):
                    ea(jt)
                    eb(jt)
            if len(weave) % 2:
                em = weave[-1]
                for jt in range(NT):
                    em(jt)

            # finale: reductions + masked NLL assembly
            rowr = cpool.tile([128, TOK], f32)
            nc.gpsimd.partition_all_reduce(rowr[:], macc[:], 128, RED.add)
            row1 = cpool.tile([1, TOK], f32)
            nc.vector.tensor_add(row1[:], rowr[0:1, :], bsel[:])
            tgts = cpool.tile([1, 1], f32)
            nc.vector.tensor_reduce(tgts[:], row1[:], AX.X, ALU.add)

            seH_r = cpool.tile([128, NT], f32)
            se0_r = cpool.tile([128, NT], f32)
            se1_r = cpool.tile([128, NT], f32)
            nc.vector.tensor_reduce(seH_r[:], seH[:], AX.X, ALU.add)
            nc.vector.tensor_reduce(se0_r[:], se0[:], AX.X, ALU.add)
            nc.vector.tensor_reduce(se1_r[:], se1[:], AX.X, ALU.add)
            logH = cpool.tile([128, NT], f32)
            log0 = cpool.tile([128, NT], f32)
            log1 = cpool.tile([128, NT], f32)
            nc.scalar.activation(logH[:], seH_r[:], AF.Ln)
            nc.scalar.activation(log0[:], se0_r[:], AF.Ln)
            nc.scalar.activation(log1[:], se1_r[:], AF.Ln)
            log0m = cpool.tile([128, NT], f32)
            log1m = cpool.tile([128, NT], f32)
            nc.vector.tensor_mul(log0m[:], log0[:], m0sb[:])
            nc.vector.tensor_mul(log1m[:], log1[:], m1sb[:])
            acc = cpool.tile([128, NT], f32)
            nc.vector.tensor_add(acc[:], logH[:], log0m[:])
            nc.vector.tensor_add(acc[:], acc[:], log1m[:])
            accr = cpool.tile([128, NT], f32)
            nc.gpsimd.partition_all_reduce(accr[:], acc[:], 128, RED.add)
            logsum = cpool.tile([1, 1], f32)
            nc.vector.tensor_reduce(logsum[:], accr[0:1, :], AX.X, ALU.add)
            res = cpool.tile([1, 1], f32)
            nc.vector.tensor_sub(res[:], logsum[:], tgts[:])
            nc.sync.dma_start(out=out_h[:], in_=res[:])

    nc.compile()
    return nc


# ---------------- entry point ----------------

def kernel(**inputs):
    global LAST_EXEC_NS
    _install_axon_profile_shim()
    from concourse import bass_utils

    w_in = np.asarray(inputs["w_in"], dtype=np.float32)
    target = np.asarray(inputs["target"], dtype=np.int64)
    head_w = np.asarray(inputs["head_w"], dtype=np.float32)
    head_b = np.asarray(inputs["head_b"], dtype=np.float32)
    t0w1 = np.asarray(inputs["tail0_w1"], dtype=np.float32)
    t0w2 = np.asarray(inputs["tail0_w2"], dtype=np.float32)
    t1w1 = np.asarray(inputs["tail1_w1"], dtype=np.float32)
    t1w2 = np.asarray(inputs["tail1_w2"], dtype=np.float32)

    # target-derived bookkeeping (pure indexing, part of input sharding)
    m0 = (target >= CUTOFF[0]) & (target < CUTOFF[1])
    m1 = (target >= CUTOFF[1]) & (target < CUTOFF[2])
    first_target = np.where(m0, CUTOFF[0], np.where(m1, CUTOFF[0] + 1, target))
    idx0 = np.clip(target - CUTOFF[0], 0, T0_V - 1)
    idx1 = np.clip(target - CUTOFF[1], 0, T1_V - 1)

    # shared (replicated) weight payloads, laid out as their SBUF images
    shared = {
        "bext": (head_b[None, :] * W8_SCALE).astype(BF16),
        "hw": _chunk_weights(head_w, H_CH, FP8, W8_SCALE),
        "w20": _chunk_weights(t0w2, T0_CH, FP8, W8_SCALE),
        "w21": _chunk_weights(t1w2, T1_CH, FP8, W8_SCALE),
        "w10": _tile_k_f8(t0w1, 32.0),
        "w11": _tile_k_f8(t1w1, 32.0),
    }

    wiT = w_in.T  # [D, N]
    selH_all = head_w[:, first_target]            # [D, N]
    sel0_all = t0w2[:, idx0] * m0[None, :]        # [D, N] masked
    sel1_all = t1w2[:, idx1] * m1[None, :]        # [D1, N] masked
    bsel_all = head_b[first_target]

    in_maps = []
    for c in range(NCORES):
        sl = slice(c * TOK, (c + 1) * TOK)
        im = dict(shared)
        im["wiT"] = _tile_k(wiT[:, sl])
        w8c = _tile_k(wiT[:, sl]).astype(FP8)  # [128, 8, 512]
        im["wiT8"] = np.ascontiguousarray(
            w8c.reshape(128, 8, NT, 128).transpose(2, 0, 1, 3))
        im["wiT8f"] = w8c
        im["selH"] = _tile_k(selH_all[:, sl])
        im["sel0"] = _tile_k(sel0_all[:, sl])
        im["sel1"] = _tile_k(sel1_all[:, sl])
        im["bsel"] = bsel_all[sl][None, :].astype(np.float32)
        im["m0"] = np.ascontiguousarray(
            m0[sl].astype(np.float32).reshape(NT, 128).T
        )
        im["m1"] = np.ascontiguousarray(
            m1[sl].astype(np.float32).reshape(NT, 128).T
        )
        in_maps.append(im)

    use_bias = bool(np.any(head_b))
    key = ("nc", use_bias)
    if key not in _CACHE:
        _CACHE[key] = _build(use_bias)
    nc = _CACHE[key]

    trace = bool(os.environ.get("BASS_TRACE"))
    res = bass_utils.run_bass_kernel_spmd(
        nc, in_maps, core_ids=list(range(NCORES)), trace=trace
    )
    LAST_EXEC_NS = res.exec_time_ns
    total = sum(float(res.results[c]["out"][0, 0]) for c in range(NCORES))
    return np.float32(total / N)


# revision 23
# speedup vs baseline: 1.1707x; 1.1707x over previous
"""Adaptive-softmax NLL on 8 TRN2 NeuronCores (Bass/Tile, SPMD data-parallel).

Strategy: shard the 4096 tokens across 8 cores (512 each). Each core computes
its tokens' full NLL (head + both tails) in bf16 on TensorE, with vocab on the
free dim and tokens on PSUM partitions; ScalarE does exp with fused free-dim
accumulation (accum_out) so the softmax denominators come out of the same pass.
Target logits are computed from host-gathered weight columns (MoE-style
dispatch done at input-prep time) as elementwise-mul + ones-matvec partition
reductions. Each core emits one partial-loss scalar; the host sums 8 scalars
and divides by N.
"""

import os
import sys
import types

import numpy as np
import ml_dtypes

BF16 = ml_dtypes.bfloat16
FP8 = ml_dtypes.float8_e4m3
W8_SCALE = 256.0

# ---- problem constants (hardcoded; kernel.py must be self-contained) ----
CUTOFF = [4000, 20000, 50000]
D = 1024
N = 4096
NCORES = 8
TOK = N // NCORES          # 512 tokens per core
NT = TOK // 128            # 4 token tiles of 128
HEAD_V = CUTOFF[0] + 2     # 4002
T0_V = CUTOFF[1] - CUTOFF[0]   # 16000
T1_V = CUTOFF[2] - CUTOFF[1]   # 30000
D1 = D // 4                # 256 tail1 bottleneck


def _chunks(v):
    out = []
    while v > 0:
        out.append(min(512, v))
        v -= out[-1]
    return out


H_CH = _chunks(HEAD_V)     # [512]*7 + [418]
T0_CH = _chunks(T0_V)      # [512]*31 + [128]
T1_CH = _chunks(T1_V)      # [512]*58 + [296]

LAST_EXEC_NS = None
_CACHE = {}


def _install_axon_profile_shim():
    """The image's antenv lacks axon_hooks; register the NTFF hook + disable
    the FishPath artifact upload so BASS_TRACE=1 profiling works locally."""
    if "antenv.axon_hooks" not in sys.modules:
        try:
            import antenv  # noqa
            mod = types.ModuleType("antenv.axon_hooks")
            _hook = [None]
            mod.set_axon_ntff_profile_hook = lambda h: _hook.__setitem__(0, h)
            mod.get_axon_ntff_profile_hook = lambda: _hook[0]
            sys.modules["antenv.axon_hooks"] = mod
            antenv.axon_hooks = mod
            from trn_agent_boot.trn_boot import _ntff_profile_via_ctypes
            mod.set_axon_ntff_profile_hook(
                _ntff_profile_via_ctypes("/opt/axon/libaxon_pjrt.so")
            )
        except Exception:
            pass
    try:
        from concourse import bass_utils
        bass_utils.upload_artifacts = lambda tmpdir: f"local:{tmpdir}"
    except Exception:
        pass


# ---------------- host-side layout helpers ----------------

def _tile_k(w):
    """[K, M] f32 -> [128, K//128, M] bf16 (partition, k-tile, free)."""
    K, M = w.shape
    kd = K // 128
    return np.ascontiguousarray(
        w.reshape(kd, 128, M).transpose(1, 0, 2)
    ).astype(BF16)


def _tile_k_f8(w, scale):
    K, M = w.shape
    kd = K // 128
    return np.ascontiguousarray(
        (w * scale).reshape(kd, 128, M).transpose(1, 0, 2)
    ).astype(FP8)


def _chunk_weights(w, chunk_sizes, dtype=BF16, scale=1.0):
    """[K, V] f32 -> [nchunk, 128, K//128, 512], zero-padded ragged."""
    K, V = w.shape
    kd = K // 128
    out = np.zeros((len(chunk_sizes), 128, kd, 512), dtype=dtype)
    c0 = 0
    for i, ncs in enumerate(chunk_sizes):
        blk = (w[:, c0:c0 + ncs] * scale).reshape(kd, 128, ncs).transpose(1, 0, 2)
        out[i, :, :, :ncs] = blk.astype(dtype)
        c0 += ncs
    return out


# ---------------- device kernel builder ----------------

H1_SCALE = 32.0  # fp8 scale for the bottleneck weights w1


def _build(use_bias):
    from concourse import bass, bacc, tile, bass_isa

    mybir = bass.mybir
    dt = mybir.dt
    bf = dt.bfloat16
    f32 = dt.float32
    f8 = dt.float8e4
    AF = mybir.ActivationFunctionType
    ALU = mybir.AluOpType
    AX = mybir.AxisListType
    DR = mybir.MatmulPerfMode.DoubleRow
    RED = bass_isa.ReduceOp

    nc = bacc.Bacc(
        "TRN2",
        target_bir_lowering=False,
        debug=False,
        enable_asserts=False,
        num_devices=NCORES,
    )

    def din(name, shape, dtype=bf):
        return nc.dram_tensor(name, list(shape), dtype, kind="ExternalInput")

    wiT_h = din("wiT", (128, 8, TOK))
    wiT8_h = din("wiT8", (NT, 128, 8, 128), dt.float8e4)
    wiT8f_h = din("wiT8f", (128, 8, TOK), dt.float8e4)
    selH_h = din("selH", (128, 8, TOK))
    sel0_h = din("sel0", (128, 8, TOK))
    sel1_h = din("sel1", (128, 2, TOK))
    bsel_h = din("bsel", (1, TOK), f32)
    m0_h = din("m0", (128, NT), f32)
    m1_h = din("m1", (128, NT), f32)
    bext_h = din("bext", (1, HEAD_V))
    hw_h = din("hw", (len(H_CH), 128, 8, 512), f8)
    w20_h = din("w20", (len(T0_CH), 128, 8, 512), f8)
    w21_h = din("w21", (len(T1_CH), 128, 2, 512), f8)
    w10_h = din("w10", (128, 8, D), f8)
    w11_h = din("w11", (128, 8, D1), f8)
    out_h = nc.dram_tensor("out", [1, 1], f32, kind="ExternalOutput")

    with tile.TileContext(nc) as tc:
        with (
            tc.tile_pool(name="const", bufs=1) as cpool,
            tc.tile_pool(name="wstream", bufs=10) as wpool,
            tc.tile_pool(name="scratch", bufs=3) as spool,
            tc.tile_pool(name="pmm", bufs=int(os.environ.get("K_PSLOTS", "2")), space=bass.MemorySpace.PSUM) as pmm,
        ):
            CPG = int(os.environ.get("K_CPG", "4"))   # chunks per macro group
            GW = 512 * CPG                             # psum banks per slot = CPG

            def groups(chunk_sizes):
                out = []
                for g0 in range(0, len(chunk_sizes), CPG):
                    cs = chunk_sizes[g0:g0 + CPG]
                    items = []
                    off = 0
                    for i, ncs in enumerate(cs):
                        items.append((g0 + i, ncs, off))
                        off += ncs
                    out.append((g0 // CPG, items, off))
                return out

            wiT = cpool.tile([128, 8, TOK], bf)
            wiT8 = []
            for j in range(NT):
                wiT8_j = cpool.tile([128, 8, 128], f8, tag=f"wiT8_{j}")
                wiT8.append(wiT8_j)
            wiT8f = cpool.tile([128, 8, TOK], f8)
            w10 = cpool.tile([128, 8, D], f8)
            w11 = cpool.tile([128, 8, D1], f8)
            selH = cpool.tile([128, 8, TOK], bf)
            sel0 = cpool.tile([128, 8, TOK], bf)
            sel1 = cpool.tile([128, 2, TOK], bf)
            bsel = cpool.tile([1, TOK], f32)
            m0sb = cpool.tile([128, NT], f32)
            m1sb = cpool.tile([128, NT], f32)
            bext = cpool.tile([1, HEAD_V], bf)
            h0T = cpool.tile([128, 8, TOK], bf)
            h1T = cpool.tile([128, 2, TOK], bf)
            h0T8 = cpool.tile([128, 8, TOK], f8)
            h1T8 = cpool.tile([128, 2, TOK], f8)
            nGH = (len(H_CH) + CPG - 1) // CPG
            nG0 = (len(T0_CH) + CPG - 1) // CPG
            nG1 = (len(T1_CH) + CPG - 1) // CPG
            seH = cpool.tile([128, NT, nGH], f32)
            se0 = cpool.tile([128, NT, nG0], f32)
            se1 = cpool.tile([128, NT, nG1], f32)
            ones_row = cpool.tile([1, 128], bf)
            macc = cpool.tile([128, TOK], f32)

            for j in range(NT):
                nc.sync.dma_start(out=wiT8[j][:], in_=wiT8_h.ap()[j])
            nc.sync.dma_start(out=wiT8f[:], in_=wiT8f_h[:])
            nc.sync.dma_start(out=bext[:], in_=bext_h[:])
            for p in range(0, 128, 32):
                nc.sync.dma_start(out=w10[p:p + 32], in_=w10_h.ap()[p:p + 32])
            nc.sync.dma_start(out=w11[:], in_=w11_h[:])
            nc.sync.dma_start(out=wiT[:], in_=wiT_h[:])
            nc.sync.dma_start(out=m0sb[:], in_=m0_h[:])
            nc.sync.dma_start(out=m1sb[:], in_=m1_h[:])
            nc.sync.dma_start(out=bsel[:], in_=bsel_h[:])
            nc.vector.memset(ones_row[:], 1.0)

            hbase = [0]
            for ncs in H_CH:
                hbase.append(hbase[-1] + ncs)

            def group_emitter(wh, nk, lhsT8, se, items, gw, bias, split=1):
                """Returns emit(jt): matmuls + exp for one token tile of one
                macro group. Weight DMAs are issued on first use."""
                nk2 = nk // 2
                g = items[0][0] // CPG
                state = {"wts": None, "split": split}

                def emit(jt):
                    if state["wts"] is None:
                        state["wts"] = []
                        for c, ncs, off in items:
                            wt = wpool.tile([128, nk, 512], f8, tag=f"w{nk}")
                            sp = 128 // state["split"]
                            for p in range(0, 128, sp):
                                nc.sync.dma_start(out=wt[p:p + sp],
                                                  in_=wh.ap()[c, p:p + sp])
                            state["wts"].append(wt)
                    ps = pmm.tile([128, GW], f32, tag="mm")
                    for (c, ncs, off), wt in zip(items, state["wts"]):
                        for k2 in range(nk2):
                            if isinstance(lhsT8, list):
                                lt = lhsT8[jt][:, 2 * k2:2 * k2 + 2, :]
                            else:
                                lt = lhsT8[:, 2 * k2:2 * k2 + 2,
                                           jt * 128:(jt + 1) * 128]
                            nc.tensor.matmul(
                                ps[:, off:off + ncs],
                                lt,
                                wt[:, 2 * k2:2 * k2 + 2, :ncs],
                                start=(k2 == 0),
                                stop=(k2 == nk2 - 1 and bias is None),
                                perf_mode=DR,
                            )
                        if bias is not None:
                            nc.tensor.matmul(
                                ps[:, off:off + ncs],
                                ones_row[:, :],
                                bias[:, hbase[c]:hbase[c] + ncs],
                                start=False,
                                stop=True,
                            )
                    nc.scalar.activation(
                        ps[:, :gw],
                        ps[:, :gw],
                        AF.Exp,
                        scale=1.0 / W8_SCALE,
                        accum_out=se[:, jt, g:g + 1],
                    )
                return emit

            def h_thunk(w1t, hT, hT8, m):
                def emit():
                    ps = pmm.tile([128, GW], f32, tag="mm")
                    for k2 in range(4):
                        nc.tensor.matmul(
                            ps[:, :TOK],
                            w1t[:, 2 * k2:2 * k2 + 2, m * 128:(m + 1) * 128],
                            wiT8f[:, 2 * k2:2 * k2 + 2, :],
                            start=(k2 == 0),
                            stop=(k2 == 3),
                            perf_mode=DR,
                        )
                    nc.vector.tensor_scalar_mul(hT[:, m, :], ps[:, :TOK],
                                                1.0 / H1_SCALE)
                    nc.vector.tensor_scalar_mul(hT8[:, m, :], ps[:, :TOK],
                                                1.0 / H1_SCALE)
                return emit

            head_groups = groups(H_CH)
            t0_groups = groups(T0_CH)
            t1_groups = groups(T1_CH)
            bias_t = bext if use_bias else None

            # phase 1: head groups jt-interleaved with h0/h1 tiles (PE-only)
            h_thunks = [h_thunk(w10, h0T, h0T8, m) for m in range(8)]
            h_thunks += [h_thunk(w11, h1T, h1T8, m) for m in range(2)]
            for gi, (g, items, gw) in enumerate(head_groups):
                em = group_emitter(hw_h, 8, wiT8, seH, items, gw, bias_t,
                                   split=4 if gi == 0 else (2 if gi == 1 else 1))
                for jt in range(NT):
                    em(jt)
                    if h_thunks:
                        h_thunks.pop(0)()
            while h_thunks:
                h_thunks.pop(0)()

            # target-logit dots on DVE (fills gaps; no PE/PSUM involved)
            nc.sync.dma_start(out=selH[:], in_=selH_h[:])
            nc.sync.dma_start(out=sel0[:], in_=sel0_h[:])
            nc.sync.dma_start(out=sel1[:], in_=sel1_h[:])
            pieces = [(wiT, selH, 8), (h0T, sel0, 8), (h1T, sel1, 2)]
            first = True
            for a, b, nk in pieces:
                for k in range(nk):
                    mt = spool.tile([128, TOK], f32, tag="mul")
                    nc.vector.tensor_mul(mt[:], a[:, k, :], b[:, k, :])
                    if first:
                        nc.vector.tensor_copy(macc[:], mt[:])
                        first = False
                    else:
                        nc.vector.tensor_add(macc[:], macc[:], mt[:])

            # phase 2: weave t0 (PE-heavy) with t1 (ACT-heavy), then pair
            # adjacent woven groups and interleave their token tiles
            weave = []
            i0, i1 = 0, 0
            n0, n1 = len(t0_groups), len(t1_groups)
            while i0 < n0 or i1 < n1:
                pick1 = (i1 < n1) and (i0 >= n0 or (n1 - i1) * n0 >= (n0 - i0) * n1)
                if pick1:
                    g, items, gw = t1_groups[i1]
                    weave.append(group_emitter(w21_h, 2, h1T8, se1, items, gw, None))
                    i1 += 1
                else:
                    g, items, gw = t0_groups[i0]
                    weave.append(group_emitter(w20_h, 8, h0T8, se0, items, gw, None))
                    i0 += 1
            for p in range(0, len(weave) - 1, 2):
                ea, eb = weave[p], weave[p + 1]
                for jt in range(NT):
                    ea(jt)
                    eb(jt)
            if len(weave) % 2:
                em = weave[-1]
                for jt in range(NT):
                    em(jt)

            # finale: reductions + masked NLL assembly
            rowr = cpool.tile([128, TOK], f32)
            nc.gpsimd.partition_all_reduce(rowr[:], macc[:], 128, RED.add)
            row1 = cpool.tile([1, TOK], f32)
            nc.vector.tensor_add(row1[:], rowr[0:1, :], bsel[:])
            tgts = cpool.tile([1, 1], f32)
            nc.vector.tensor_reduce(tgts[:], row1[:], AX.X, ALU.add)

            seH_r = cpool.tile([128, NT], f32)
            se0_r = cpool.tile([128, NT], f32)
            se1_r = cpool.tile([128, NT], f32)
            nc.vector.tensor_reduce(seH_r[:], seH[:], AX.X, ALU.add)
            nc.vector.tensor_reduce(se0_r[:], se0[:], AX.X, ALU.add)
            nc.vector.tensor_reduce(se1_r[:], se1[:], AX.X, ALU.add)
            logH = cpool.tile([128, NT], f32)
            log0 = cpool.tile([128, NT], f32)
            log1 = cpool.tile([128, NT], f32)
            nc.scalar.activation(logH[:], seH_r[:], AF.Ln)
            nc.scalar.activation(log0[:], se0_r[:], AF.Ln)
            nc.scalar.activation(log1[:], se1_r[:], AF.Ln)
            log0m = cpool.tile([128, NT], f32)
            log1m = cpool.tile([128, NT], f32)
            nc.vector.tensor_mul(log0m[:], log0[:], m0sb[:])
            nc.vector.tensor_mul(log1m[:], log1[:], m1sb[:])
            acc = cpool.tile([128, NT], f32)
            nc.vector.tensor_add(acc[:], logH[:], log0m[:])
            nc.vector.tensor_add(acc[:], acc[:], log1m[:])
            accr = cpool.tile([128, NT], f32)
            nc.gpsimd.partition_all_reduce(accr[:], acc[:], 128, RED.add)
            logsum = cpool.tile([1, 1], f32)
            nc.vector.tensor_reduce(logsum[:], accr[0:1, :], AX.X, ALU.add)
            res = cpool.tile([1, 1], f32)
            nc.vector.tensor_sub(res[:], logsum[:], tgts[:])
            nc.sync.dma_start(out=out_h[:], in_=res[:])

    nc.compile()
    return nc


# ---------------- entry point ----------------

def kernel(**inputs):
    global LAST_EXEC_NS
    _install_axon_profile_shim()
    from concourse import bass_utils

    w_in = np.asarray(inputs["w_in"], dtype=np.float32)
    target = np.asarray(inputs["target"], dtype=np.int64)
    head_w = np.asarray(inputs["head_w"], dtype=np.float32)
    head_b = np.asarray(inputs["head_b"], dtype=np.float32)
    t0w1 = np.asarray(inputs["tail0_w1"], dtype=np.float32)
    t0w2 = np.asarray(inputs["tail0_w2"], dtype=np.float32)
    t1w1 = np.asarray(inputs["tail1_w1"], dtype=np.float32)
    t1w2 = np.asarray(inputs["tail1_w2"], dtype=np.float32)

    # target-derived bookkeeping (pure indexing, part of input sharding)
    m0 = (target >= CUTOFF[0]) & (target < CUTOFF[1])
    m1 = (target >= CUTOFF[1]) & (target < CUTOFF[2])
    first_target = np.where(m0, CUTOFF[0], np.where(m1, CUTOFF[0] + 1, target))
    idx0 = np.clip(target - CUTOFF[0], 0, T0_V - 1)
    idx1 = np.clip(target - CUTOFF[1], 0, T1_V - 1)

    # shared (replicated) weight payloads, laid out as their SBUF images
    shared = {
        "bext": (head_b[None, :] * W8_SCALE).astype(BF16),
        "hw": _chunk_weights(head_w, H_CH, FP8, W8_SCALE),
        "w20": _chunk_weights(t0w2, T0_CH, FP8, W8_SCALE),
        "w21": _chunk_weights(t1w2, T1_CH, FP8, W8_SCALE),
        "w10": _tile_k_f8(t0w1, 32.0),
        "w11": _tile_k_f8(t1w1, 32.0),
    }

    wiT = w_in.T  # [D, N]
    selH_all = head_w[:, first_target]            # [D, N]
    sel0_all = t0w2[:, idx0] * m0[None, :]        # [D, N] masked
    sel1_all = t1w2[:, idx1] * m1[None, :]        # [D1, N] masked
    bsel_all = head_b[first_target]

    in_maps = []
    for c in range(NCORES):
        sl = slice(c * TOK, (c + 1) * TOK)
        im = dict(shared)
        im["wiT"] = _tile_k(wiT[:, sl])
        w8c = _tile_k(wiT[:, sl]).astype(FP8)  # [128, 8, 512]
        im["wiT8"] = np.ascontiguousarray(
            w8c.reshape(128, 8, NT, 128).transpose(2, 0, 1, 3))
        im["wiT8f"] = w8c
        im["selH"] = _tile_k(selH_all[:, sl])
        im["sel0"] = _tile_k(sel0_all[:, sl])
        im["sel1"] = _tile_k(sel1_all[:, sl])
        im["bsel"] = bsel_all[sl][None, :].astype(np.float32)
        im["m0"] = np.ascontiguousarray(
            m0[sl].astype(np.float32).reshape(NT, 128).T
        )
        im["m1"] = np.ascontiguousarray(
            m1[sl].astype(np.float32).reshape(NT, 128).T
        )
        in_maps.append(im)

    use_bias = bool(np.any(head_b))
    key = ("nc", use_bias)
    if key not in _CACHE:
        _CACHE[key] = _build(use_bias)
    nc = _CACHE[key]

    trace = bool(os.environ.get("BASS_TRACE"))
    res = bass_utils.run_bass_kernel_spmd(
        nc, in_maps, core_ids=list(range(NCORES)), trace=trace
    )
    LAST_EXEC_NS = res.exec_time_ns
    total = sum(float(res.results[c]["out"][0, 0]) for c in range(NCORES))
    return np.float32(total / N)


# revision 24
# speedup vs baseline: 1.1753x; 1.0039x over previous
"""Adaptive-softmax NLL on 8 TRN2 NeuronCores (Bass/Tile, SPMD data-parallel).

Strategy: shard the 4096 tokens across 8 cores (512 each). Each core computes
its tokens' full NLL (head + both tails) in bf16 on TensorE, with vocab on the
free dim and tokens on PSUM partitions; ScalarE does exp with fused free-dim
accumulation (accum_out) so the softmax denominators come out of the same pass.
Target logits are computed from host-gathered weight columns (MoE-style
dispatch done at input-prep time) as elementwise-mul + ones-matvec partition
reductions. Each core emits one partial-loss scalar; the host sums 8 scalars
and divides by N.
"""

import os
import sys
import types

import numpy as np
import ml_dtypes

BF16 = ml_dtypes.bfloat16
FP8 = ml_dtypes.float8_e4m3
W8_SCALE = 256.0

# ---- problem constants (hardcoded; kernel.py must be self-contained) ----
CUTOFF = [4000, 20000, 50000]
D = 1024
N = 4096
NCORES = 8
TOK = N // NCORES          # 512 tokens per core
NT = TOK // 128            # 4 token tiles of 128
HEAD_V = CUTOFF[0] + 2     # 4002
T0_V = CUTOFF[1] - CUTOFF[0]   # 16000
T1_V = CUTOFF[2] - CUTOFF[1]   # 30000
D1 = D // 4                # 256 tail1 bottleneck


def _chunks(v):
    out = []
    while v > 0:
        out.append(min(512, v))
        v -= out[-1]
    return out


H_CH = _chunks(HEAD_V)     # [512]*7 + [418]
T0_CH = _chunks(T0_V)      # [512]*31 + [128]
T1_CH = _chunks(T1_V)      # [512]*58 + [296]

LAST_EXEC_NS = None
_CACHE = {}


def _install_axon_profile_shim():
    """The image's antenv lacks axon_hooks; register the NTFF hook + disable
    the FishPath artifact upload so BASS_TRACE=1 profiling works locally."""
    if "antenv.axon_hooks" not in sys.modules:
        try:
            import antenv  # noqa
            mod = types.ModuleType("antenv.axon_hooks")
            _hook = [None]
            mod.set_axon_ntff_profile_hook = lambda h: _hook.__setitem__(0, h)
            mod.get_axon_ntff_profile_hook = lambda: _hook[0]
            sys.modules["antenv.axon_hooks"] = mod
            antenv.axon_hooks = mod
            from trn_agent_boot.trn_boot import _ntff_profile_via_ctypes
            mod.set_axon_ntff_profile_hook(
                _ntff_profile_via_ctypes("/opt/axon/libaxon_pjrt.so")
            )
        except Exception:
            pass
    try:
        from concourse import bass_utils
        bass_utils.upload_artifacts = lambda tmpdir: f"local:{tmpdir}"
    except Exception:
        pass


# ---------------- host-side layout helpers ----------------

def _tile_k(w):
    """[K, M] f32 -> [128, K//128, M] bf16 (partition, k-tile, free)."""
    K, M = w.shape
    kd = K // 128
    return np.ascontiguousarray(
        w.reshape(kd, 128, M).transpose(1, 0, 2)
    ).astype(BF16)


def _tile_k_f8(w, scale):
    K, M = w.shape
    kd = K // 128
    return np.ascontiguousarray(
        (w * scale).reshape(kd, 128, M).transpose(1, 0, 2)
    ).astype(FP8)


def _chunk_weights(w, chunk_sizes, dtype=BF16, scale=1.0):
    """[K, V] f32 -> [nchunk, 128, K//128, 512], zero-padded ragged."""
    K, V = w.shape
    kd = K // 128
    out = np.zeros((len(chunk_sizes), 128, kd, 512), dtype=dtype)
    c0 = 0
    for i, ncs in enumerate(chunk_sizes):
        blk = (w[:, c0:c0 + ncs] * scale).reshape(kd, 128, ncs).transpose(1, 0, 2)
        out[i, :, :, :ncs] = blk.astype(dtype)
        c0 += ncs
    return out


# ---------------- device kernel builder ----------------

H1_SCALE = 32.0  # fp8 scale for the bottleneck weights w1


def _build(use_bias):
    from concourse import bass, bacc, tile, bass_isa

    mybir = bass.mybir
    dt = mybir.dt
    bf = dt.bfloat16
    f32 = dt.float32
    f8 = dt.float8e4
    AF = mybir.ActivationFunctionType
    ALU = mybir.AluOpType
    AX = mybir.AxisListType
    DR = mybir.MatmulPerfMode.DoubleRow
    RED = bass_isa.ReduceOp

    nc = bacc.Bacc(
        "TRN2",
        target_bir_lowering=False,
        debug=False,
        enable_asserts=False,
        num_devices=NCORES,
    )

    def din(name, shape, dtype=bf):
        return nc.dram_tensor(name, list(shape), dtype, kind="ExternalInput")

    wiT_h = din("wiT", (128, 8, TOK))
    wiT8_h = din("wiT8", (128, 8, TOK), dt.float8e4)
    selH_h = din("selH", (128, 8, TOK))
    sel0_h = din("sel0", (128, 8, TOK))
    sel1_h = din("sel1", (128, 2, TOK))
    bsel_h = din("bsel", (1, TOK), f32)
    m0_h = din("m0", (128, NT), f32)
    m1_h = din("m1", (128, NT), f32)
    bext_h = din("bext", (1, HEAD_V))
    hw_h = din("hw", (len(H_CH), 128, 8, 512), f8)
    w20_h = din("w20", (len(T0_CH), 128, 8, 512), f8)
    w21_h = din("w21", (len(T1_CH), 128, 2, 512), f8)
    w10_h = din("w10", (128, 8, D), f8)
    w11_h = din("w11", (128, 8, D1), f8)
    out_h = nc.dram_tensor("out", [1, 1], f32, kind="ExternalOutput")

    with tile.TileContext(nc) as tc:
        with (
            tc.tile_pool(name="const", bufs=1) as cpool,
            tc.tile_pool(name="wstream", bufs=10) as wpool,
            tc.tile_pool(name="scratch", bufs=3) as spool,
            tc.tile_pool(name="pmm", bufs=int(os.environ.get("K_PSLOTS", "2")), space=bass.MemorySpace.PSUM) as pmm,
        ):
            CPG = int(os.environ.get("K_CPG", "4"))   # chunks per macro group
            GW = 512 * CPG                             # psum banks per slot = CPG

            def groups(chunk_sizes):
                out = []
                for g0 in range(0, len(chunk_sizes), CPG):
                    cs = chunk_sizes[g0:g0 + CPG]
                    items = []
                    off = 0
                    for i, ncs in enumerate(cs):
                        items.append((g0 + i, ncs, off))
                        off += ncs
                    out.append((g0 // CPG, items, off))
                return out

            wiT = cpool.tile([128, 8, TOK], bf)
            wiT8 = cpool.tile([128, 8, TOK], f8)
            w10 = cpool.tile([128, 8, D], f8)
            w11 = cpool.tile([128, 8, D1], f8)
            selH = cpool.tile([128, 8, TOK], bf)
            sel0 = cpool.tile([128, 8, TOK], bf)
            sel1 = cpool.tile([128, 2, TOK], bf)
            bsel = cpool.tile([1, TOK], f32)
            m0sb = cpool.tile([128, NT], f32)
            m1sb = cpool.tile([128, NT], f32)
            bext = cpool.tile([1, HEAD_V], bf)
            h0T = cpool.tile([128, 8, TOK], bf)
            h1T = cpool.tile([128, 2, TOK], bf)
            h0T8 = cpool.tile([128, 8, TOK], f8)
            h1T8 = cpool.tile([128, 2, TOK], f8)
            nGH = (len(H_CH) + CPG - 1) // CPG
            nG0 = (len(T0_CH) + CPG - 1) // CPG
            nG1 = (len(T1_CH) + CPG - 1) // CPG
            seH = cpool.tile([128, NT, nGH], f32)
            se0 = cpool.tile([128, NT, nG0], f32)
            se1 = cpool.tile([128, NT, nG1], f32)
            ones_row = cpool.tile([1, 128], bf)
            macc = cpool.tile([128, TOK], f32)

            for p in range(0, 128, 32):
                nc.sync.dma_start(out=wiT8[p:p + 32], in_=wiT8_h.ap()[p:p + 32])
            nc.sync.dma_start(out=bext[:], in_=bext_h[:])
            for p in range(0, 128, 32):
                nc.sync.dma_start(out=w10[p:p + 32], in_=w10_h.ap()[p:p + 32])
            nc.sync.dma_start(out=w11[:], in_=w11_h[:])
            nc.sync.dma_start(out=wiT[:], in_=wiT_h[:])
            nc.sync.dma_start(out=m0sb[:], in_=m0_h[:])
            nc.sync.dma_start(out=m1sb[:], in_=m1_h[:])
            nc.sync.dma_start(out=bsel[:], in_=bsel_h[:])
            nc.vector.memset(ones_row[:], 1.0)

            hbase = [0]
            for ncs in H_CH:
                hbase.append(hbase[-1] + ncs)

            def group_emitter(wh, nk, lhsT8, se, items, gw, bias, split=1):
                """Returns emit(jt): matmuls + exp for one token tile of one
                macro group. Weight DMAs are issued on first use."""
                nk2 = nk // 2
                g = items[0][0] // CPG
                state = {"wts": None, "split": split}

                def emit(jt):
                    if state["wts"] is None:
                        state["wts"] = []
                        for c, ncs, off in items:
                            wt = wpool.tile([128, nk, 512], f8, tag=f"w{nk}")
                            sp = 128 // state["split"]
                            for p in range(0, 128, sp):
                                nc.sync.dma_start(out=wt[p:p + sp],
                                                  in_=wh.ap()[c, p:p + sp])
                            state["wts"].append(wt)
                    ps = pmm.tile([128, GW], f32, tag="mm")
                    for (c, ncs, off), wt in zip(items, state["wts"]):
                        for k2 in range(nk2):
                            lt = lhsT8[:, 2 * k2:2 * k2 + 2,
                                       jt * 128:(jt + 1) * 128]
                            nc.tensor.matmul(
                                ps[:, off:off + ncs],
                                lt,
                                wt[:, 2 * k2:2 * k2 + 2, :ncs],
                                start=(k2 == 0),
                                stop=(k2 == nk2 - 1 and bias is None),
                                perf_mode=DR,
                            )
                        if bias is not None:
                            nc.tensor.matmul(
                                ps[:, off:off + ncs],
                                ones_row[:, :],
                                bias[:, hbase[c]:hbase[c] + ncs],
                                start=False,
                                stop=True,
                            )
                    nc.scalar.activation(
                        ps[:, :gw],
                        ps[:, :gw],
                        AF.Exp,
                        scale=1.0 / W8_SCALE,
                        accum_out=se[:, jt, g:g + 1],
                    )
                return emit

            def h_thunk(w1t, hT, hT8, m):
                def emit():
                    ps = pmm.tile([128, GW], f32, tag="mm")
                    for k2 in range(4):
                        nc.tensor.matmul(
                            ps[:, :TOK],
                            w1t[:, 2 * k2:2 * k2 + 2, m * 128:(m + 1) * 128],
                            wiT8[:, 2 * k2:2 * k2 + 2, :],
                            start=(k2 == 0),
                            stop=(k2 == 3),
                            perf_mode=DR,
                        )
                    nc.vector.tensor_scalar_mul(hT[:, m, :], ps[:, :TOK],
                                                1.0 / H1_SCALE)
                    nc.vector.tensor_scalar_mul(hT8[:, m, :], ps[:, :TOK],
                                                1.0 / H1_SCALE)
                return emit

            head_groups = groups(H_CH)
            t0_groups = groups(T0_CH)
            t1_groups = groups(T1_CH)
            bias_t = bext if use_bias else None

            # phase 1: head groups jt-interleaved with h0/h1 tiles (PE-only)
            h_thunks = [h_thunk(w10, h0T, h0T8, m) for m in range(8)]
            h_thunks += [h_thunk(w11, h1T, h1T8, m) for m in range(2)]
            for gi, (g, items, gw) in enumerate(head_groups):
                em = group_emitter(hw_h, 8, wiT8, seH, items, gw, bias_t,
                                   split=4 if gi == 0 else (2 if gi == 1 else 1))
                for jt in range(NT):
                    em(jt)
                    if h_thunks:
                        h_thunks.pop(0)()
            while h_thunks:
                h_thunks.pop(0)()

            # target-logit dots on DVE (fills gaps; no PE/PSUM involved)
            nc.sync.dma_start(out=selH[:], in_=selH_h[:])
            nc.sync.dma_start(out=sel0[:], in_=sel0_h[:])
            nc.sync.dma_start(out=sel1[:], in_=sel1_h[:])
            pieces = [(wiT, selH, 8), (h0T, sel0, 8), (h1T, sel1, 2)]
            first = True
            for a, b, nk in pieces:
                for k in range(nk):
                    mt = spool.tile([128, TOK], f32, tag="mul")
                    nc.vector.tensor_mul(mt[:], a[:, k, :], b[:, k, :])
                    if first:
                        nc.vector.tensor_copy(macc[:], mt[:])
                        first = False
                    else:
                        nc.vector.tensor_add(macc[:], macc[:], mt[:])

            # phase 2: weave t0 (PE-heavy) with t1 (ACT-heavy), then pair
            # adjacent woven groups and interleave their token tiles
            weave = []
            i0, i1 = 0, 0
            n0, n1 = len(t0_groups), len(t1_groups)
            while i0 < n0 or i1 < n1:
                pick1 = (i1 < n1) and (i0 >= n0 or (n1 - i1) * n0 >= (n0 - i0) * n1)
                if pick1:
                    g, items, gw = t1_groups[i1]
                    weave.append(group_emitter(w21_h, 2, h1T8, se1, items, gw, None))
                    i1 += 1
                else:
                    g, items, gw = t0_groups[i0]
                    weave.append(group_emitter(w20_h, 8, h0T8, se0, items, gw, None))
                    i0 += 1
            for p in range(0, len(weave) - 1, 2):
                ea, eb = weave[p], weave[p + 1]
                for jt in range(NT):
                    ea(jt)
                    eb(jt)
            if len(weave) % 2:
                em = weave[-1]
                for jt in range(NT):
                    em(jt)

            # finale: reductions + masked NLL assembly
            rowr = cpool.tile([128, TOK], f32)
            nc.gpsimd.partition_all_reduce(rowr[:], macc[:], 128, RED.add)
            row1 = cpool.tile([1, TOK], f32)
            nc.vector.tensor_add(row1[:], rowr[0:1, :], bsel[:])
            tgts = cpool.tile([1, 1], f32)
            nc.vector.tensor_reduce(tgts[:], row1[:], AX.X, ALU.add)

            seH_r = cpool.tile([128, NT], f32)
            se0_r = cpool.tile([128, NT], f32)
            se1_r = cpool.tile([128, NT], f32)
            nc.vector.tensor_reduce(seH_r[:], seH[:], AX.X, ALU.add)
            nc.vector.tensor_reduce(se0_r[:], se0[:], AX.X, ALU.add)
            nc.vector.tensor_reduce(se1_r[:], se1[:], AX.X, ALU.add)
            logH = cpool.tile([128, NT], f32)
            log0 = cpool.tile([128, NT], f32)
            log1 = cpool.tile([128, NT], f32)
            nc.scalar.activation(logH[:], seH_r[:], AF.Ln)
            nc.scalar.activation(log0[:], se0_r[:], AF.Ln)
            nc.scalar.activation(log1[:], se1_r[:], AF.Ln)
            log0m = cpool.tile([128, NT], f32)
            log1m = cpool.tile([128, NT], f32)
            nc.vector.tensor_mul(log0m[:], log0[:], m0sb[:])
            nc.vector.tensor_mul(log1m[:], log1[:], m1sb[:])
            acc = cpool.tile([128, NT], f32)
            nc.vector.tensor_add(acc[:], logH[:], log0m[:])
            nc.vector.tensor_add(acc[:], acc[:], log1m[:])
            accr = cpool.tile([128, NT], f32)
            nc.gpsimd.partition_all_reduce(accr[:], acc[:], 128, RED.add)
            logsum = cpool.tile([1, 1], f32)
            nc.vector.tensor_reduce(logsum[:], accr[0:1, :], AX.X, ALU.add)
            res = cpool.tile([1, 1], f32)
            nc.vector.tensor_sub(res[:], logsum[:], tgts[:])
            nc.sync.dma_start(out=out_h[:], in_=res[:])

    nc.compile()
    return nc


# ---------------- entry point ----------------

def kernel(**inputs):
    global LAST_EXEC_NS
    _install_axon_profile_shim()
    from concourse import bass_utils

    w_in = np.asarray(inputs["w_in"], dtype=np.float32)
    target = np.asarray(inputs["target"], dtype=np.int64)
    head_w = np.asarray(inputs["head_w"], dtype=np.float32)
    head_b = np.asarray(inputs["head_b"], dtype=np.float32)
    t0w1 = np.asarray(inputs["tail0_w1"], dtype=np.float32)
    t0w2 = np.asarray(inputs["tail0_w2"], dtype=np.float32)
    t1w1 = np.asarray(inputs["tail1_w1"], dtype=np.float32)
    t1w2 = np.asarray(inputs["tail1_w2"], dtype=np.float32)

    # target-derived bookkeeping (pure indexing, part of input sharding)
    m0 = (target >= CUTOFF[0]) & (target < CUTOFF[1])
    m1 = (target >= CUTOFF[1]) & (target < CUTOFF[2])
    first_target = np.where(m0, CUTOFF[0], np.where(m1, CUTOFF[0] + 1, target))
    idx0 = np.clip(target - CUTOFF[0], 0, T0_V - 1)
    idx1 = np.clip(target - CUTOFF[1], 0, T1_V - 1)

    # shared (replicated) weight payloads, laid out as their SBUF images
    shared = {
        "bext": (head_b[None, :] * W8_SCALE).astype(BF16),
        "hw": _chunk_weights(head_w, H_CH, FP8, W8_SCALE),
        "w20": _chunk_weights(t0w2, T0_CH, FP8, W8_SCALE),
        "w21": _chunk_weights(t1w2, T1_CH, FP8, W8_SCALE),
        "w10": _tile_k_f8(t0w1, 32.0),
        "w11": _tile_k_f8(t1w1, 32.0),
    }

    wiT = w_in.T  # [D, N]
    selH_all = head_w[:, first_target]            # [D, N]
    sel0_all = t0w2[:, idx0] * m0[None, :]        # [D, N] masked
    sel1_all = t1w2[:, idx1] * m1[None, :]        # [D1, N] masked
    bsel_all = head_b[first_target]

    in_maps = []
    for c in range(NCORES):
        sl = slice(c * TOK, (c + 1) * TOK)
        im = dict(shared)
        im["wiT"] = _tile_k(wiT[:, sl])
        im["wiT8"] = _tile_k(wiT[:, sl]).astype(FP8)
        im["selH"] = _tile_k(selH_all[:, sl])
        im["sel0"] = _tile_k(sel0_all[:, sl])
        im["sel1"] = _tile_k(sel1_all[:, sl])
        im["bsel"] = bsel_all[sl][None, :].astype(np.float32)
        im["m0"] = np.ascontiguousarray(
            m0[sl].astype(np.float32).reshape(NT, 128).T
        )
        im["m1"] = np.ascontiguousarray(
            m1[sl].astype(np.float32).reshape(NT, 128).T
        )
        in_maps.append(im)

    use_bias = bool(np.any(head_b))
    key = ("nc", use_bias)
    if key not in _CACHE:
        _CACHE[key] = _build(use_bias)
    nc = _CACHE[key]

    trace = bool(os.environ.get("BASS_TRACE"))
    res = bass_utils.run_bass_kernel_spmd(
        nc, in_maps, core_ids=list(range(NCORES)), trace=trace
    )
    LAST_EXEC_NS = res.exec_time_ns
    total = sum(float(res.results[c]["out"][0, 0]) for c in range(NCORES))
    return np.float32(total / N)
